# revision 33
# baseline (speedup 1.0000x reference)
"""Trainium2 Bass kernel for nn_Attention (B=2,T=8,N=512,C=768,H=12).

Strategy: data-parallel over the 16 (b,t) slices -> 2 slices per core, 8 cores.
All transposes and dtype conversion done on host (free). On-chip per slice:
  xT[c, n] arrives pre-transposed, bf16
  qkT[d, n] = W_qk @ xT      (bf16; scale folded into Wq on host)
  v[m, (h, dh+1)] = xT.T @ W_v  with ones column per head (bf16)
  ST[m, n] = kT.T @ qT       (bf16; no mask on PE; two m-chunks share one
                              [128,1024] PSUM pair-tile)
  P0 = exp(ST)               (Act engine, 2 wide exps per head)
  P = P0 * exp(maskT)        (multiplicative mask, Pool, SBUF bf16)
  O[n, (dh+1)] per n-tile = P.T-chunks @ v  (flipped PV, ap=65 bf16
                              matmuls; col dh accumulates l via ones column)
  Onorm = O * 1/l            (per-partition tensor_scalar on DVE)
  outT[d, n] = transpose(Onorm) on PE; y = outT.T @ proj_wT (bf16)
Attention pipeline is software-pipelined: scores(h) | PV(h-1) | transpose(h-2)
so the PE never waits on the Act exp or DVE normalize round-trips; qkv/proj
units are interleaved as PE filler throughout both slices' head phases.
"""
import sys

sys.path.insert(0, "/opt/trn_rl_repo")

import numpy as np
import ml_dtypes
import concourse.bacc as bacc
import concourse.mybir as mybir
import concourse.tile as tile
from concourse.bass_utils import run_bass_kernel_spmd
from concourse.masks import make_identity

B, T, N, C = 2, 8, 512, 768
H = 12
Dh = C // H            # 64
SL = 2                 # slices per core
NCORES = 8
NC4 = N // 128         # 4 n-chunks
CC6 = C // 128         # 6 c-chunks
F32 = mybir.dt.float32
BF16 = mybir.dt.bfloat16

_cache = {}


def build_nc():
    nc = bacc.Bacc()
    xTs = nc.dram_tensor("xTs", [SL, C, N], BF16, kind="ExternalInput")
    qkv_wTqk = nc.dram_tensor("qkv_wTqk", [C, 2 * C], BF16, kind="ExternalInput")
    qkv_wTv = nc.dram_tensor("qkv_wTv", [C, C], BF16, kind="ExternalInput")
    proj_wT = nc.dram_tensor("proj_wT", [C, C], BF16, kind="ExternalInput")
    # exp(mask.T) with m-chunk pairs side by side: row-block a holds
    # [E[256a:256a+128], E[256a+128:256a+256]] as a (128, 1024) tile
    emaskT2 = nc.dram_tensor("emaskT2", [2 * 128, 2 * N], BF16, kind="ExternalInput")
    y = nc.dram_tensor("y", [SL, N, C], BF16, kind="ExternalOutput")

    with tile.TileContext(nc) as tc:
        with (
            tc.tile_pool(name="wpool", bufs=1) as wpool,
            tc.tile_pool(name="sb", bufs=1) as sb,
            tc.tile_pool(name="ps", bufs=1, space="PSUM") as ps,
        ):
            # ---- persistent weights ----
            qkw = [wpool.tile([128, 2 * C], BF16, tag=f"qkw{cc}", name=f"qkw{cc}") for cc in range(CC6)]
            vw = [wpool.tile([128, C], BF16, tag=f"vw{cc}", name=f"vw{cc}") for cc in range(CC6)]
            projw = [wpool.tile([128, C], BF16, tag=f"projw{cc}", name=f"projw{cc}") for cc in range(CC6)]
            emask2 = [wpool.tile([128, 2 * N], BF16, tag=f"emask{a}", name=f"emask{a}") for a in range(2)]

            def emit_weight_dmas():
                # xT+vw interleaved across all three DMA queues (the v phase
                # needs every chunk of both); qkw behind them; emask last
                for cc in range(CC6):
                    eng = (nc.gpsimd, nc.sync, nc.scalar)[cc % 3]
                    eng.dma_start(vw[cc][:], qkv_wTv[128 * cc:128 * (cc + 1), :])
                for cc in range(CC6):
                    eng = nc.sync if cc < 3 else nc.gpsimd
                    eng.dma_start(qkw[cc][:], qkv_wTqk[128 * cc:128 * (cc + 1), :])
                for a in range(2):
                    nc.gpsimd.dma_start(emask2[a][:], emaskT2[128 * a:128 * (a + 1), :])

            def emit_projw_dmas():
                for cc in range(CC6):
                    nc.gpsimd.dma_start(projw[cc][:], proj_wT[128 * cc:128 * (cc + 1), :])

            onesf = wpool.tile([128, Dh], F32, tag="onesf")
            nc.gpsimd.memset(onesf[:], 1.0)
            identf = wpool.tile([128, 128], F32, tag="identf")
            make_identity(nc, identf[:])
            identb = wpool.tile([128, 128], BF16, tag="identb")
            with nc.allow_low_precision(reason="bf16 identity"):
                nc.vector.tensor_copy(identb[:], identf[:])

            xTt = [[None] * CC6 for _ in range(SL)]
            vsbs = [[None] * NC4 for _ in range(SL)]
            qks = [[None] * (2 * CC6) for _ in range(SL)]
            outTs = [[None] * CC6 for _ in range(SL)]
            ptss = [[None] * H for _ in range(SL)]   # per-head [ptA, ptB]

            def get(lst, i, mk):
                if lst[i] is None:
                    lst[i] = mk()
                return lst[i]

            def emit_x_dma(s, cc):
                xT = get(xTt[s], cc, lambda cc=cc: sb.tile(
                    [128, N], BF16, tag="xT", name=f"xT_s{s}_{cc}", bufs=12))
                if s == 0:
                    eng = (nc.sync, nc.scalar, nc.gpsimd)[cc % 3]
                else:
                    eng = nc.sync if cc % 2 == 0 else nc.gpsimd
                eng.dma_start(xT[:], xTs[s, 128 * cc:128 * (cc + 1), :])

            def emit_v(s, m4):
                xT = xTt[s]
                vsb = get(vsbs[s], m4, lambda: sb.tile(
                    [128, H * (Dh + 1)], BF16, tag="vsb", name=f"vsb_s{s}_{m4}", bufs=8))
                pva = ps.tile([128, 512], F32, tag="ps1", name=f"pva{s}_{m4}", bufs=3)
                pvb = ps.tile([128, 256], F32, tag="ps1", name=f"pvb{s}_{m4}", bufs=3)
                # pva group completes first so its copy overlaps the pvb group
                for i in range(CC6):
                    cc = (m4 + i) % CC6
                    nc.tensor.matmul(pva[:], xT[cc][:, 128 * m4:128 * (m4 + 1)],
                                     vw[cc][:, 0:512],
                                     start=(i == 0), stop=(i == CC6 - 1))
                for i in range(CC6):
                    cc = (m4 + i) % CC6
                    nc.tensor.matmul(pvb[:], xT[cc][:, 128 * m4:128 * (m4 + 1)],
                                     vw[cc][:, 512:768],
                                     start=(i == 0), stop=(i == CC6 - 1))
                v3 = vsb[:].rearrange("p (h e) -> p h e", e=Dh + 1)
                cpy = nc.scalar.copy
                with nc.allow_low_precision(reason="bf16 v"):
                    cpy(v3[:, 0:8, 0:Dh], pva[:].rearrange("p (h e) -> p h e", e=Dh))
                    cpy(v3[:, 8:12, 0:Dh], pvb[:].rearrange("p (h e) -> p h e", e=Dh))
                    nc.vector.tensor_copy(v3[:, :, Dh:Dh + 1],
                                          onesf[:, 0:H].rearrange("p (h e) -> p h e", e=1))

            def emit_qk(s, jc):
                xT = xTt[s]
                qkt = get(qks[s], jc, lambda: sb.tile(
                    [128, N], BF16, tag="qk", name=f"qk_s{s}_{jc}", bufs=13))
                pqk = ps.tile([128, N], F32, tag="ps1", name=f"pqk{s}_{jc}", bufs=3)
                for i in range(CC6):
                    cc = (jc + i) % CC6
                    nc.tensor.matmul(pqk[:], qkw[cc][:, 128 * jc:128 * (jc + 1)], xT[cc][:],
                                     start=(i == 0), stop=(i == CC6 - 1))
                cpy = nc.vector.tensor_copy if jc % 2 == 0 else nc.scalar.copy
                with nc.allow_low_precision(reason="bf16 qk"):
                    cpy(qkt[:], pqk[:])

            def emit_scores(s, h):
                """Scores for head h: two [128,1024] PSUM pair-tiles, each
                covering two m-chunks; one exp + one mask-mul per pair."""
                qk = qks[s]
                hb = 64 * (h % 2)
                qTh = qk[h // 2][hb:hb + 64, :]
                kTh = qk[CC6 + h // 2][hb:hb + 64, :]
                pts = []
                for a in range(2):
                    pst2 = ps.tile([128, 2 * N], F32, tag="ps2", name=f"pst{s}_{h}_{a}", bufs=2)
                    p02 = sb.tile([128, 2 * N], BF16, tag="p0", name=f"p0_{s}_{h}_{a}", bufs=6)
                    pt2 = sb.tile([128, 2 * N], BF16, tag="pt", name=f"pt{s}_{h}_{a}", bufs=6)
                    for b in range(2):
                        mc = 2 * a + b
                        nc.tensor.matmul(pst2[:, 512 * b:512 * (b + 1)],
                                         kTh[:, 128 * mc:128 * (mc + 1)], qTh,
                                         start=True, stop=True)
                    with nc.allow_low_precision(reason="bf16 P"):
                        nc.scalar.activation(p02[:], pst2[:],
                                             mybir.ActivationFunctionType.Exp)
                        # Pool is the lightest engine; DVE carries the
                        # normalize/copy chain
                        nc.gpsimd.tensor_mul(pt2[:], p02[:], emask2[a][:])
                    pts.append(pt2)
                ptss[s][h] = pts

            onrms = [[None] * H for _ in range(SL)]

            def emit_pv_mm(s, h):
                """Flipped PV: out[n, dh+1] per n-tile (ap=65 matmuls), then
                per-partition softmax normalize into SBUF bf16."""
                vsb = vsbs[s]
                pts = ptss[s][h]
                pot = ps.tile([128, NC4 * (Dh + 1)], F32, tag="ps1",
                              name=f"pot{s}_{h}", bufs=3)
                p3 = pot[:].rearrange("p (t e) -> p t e", e=Dh + 1)
                for nt in range(NC4):
                    for mc in range(NC4):
                        lhsT = pts[mc // 2][:, 512 * (mc % 2) + 128 * nt:
                                            512 * (mc % 2) + 128 * (nt + 1)]
                        nc.tensor.matmul(p3[:, nt, :], lhsT,
                                         vsb[mc][:, (Dh + 1) * h:(Dh + 1) * (h + 1)],
                                         start=(mc == 0), stop=(mc == NC4 - 1))
                recip4 = sb.tile([128, NC4], F32, tag="recip", name=f"recip{s}_{h}", bufs=4)
                r3 = recip4[:].rearrange("p (t e) -> p t e", e=1)
                nc.vector.reciprocal(r3[:], p3[:, :, Dh:Dh + 1])
                onrm = get(onrms[s], h, lambda: sb.tile(
                    [128, NC4 * Dh], BF16, tag="onrm", name=f"onrm{s}_{h}", bufs=4))
                with nc.allow_low_precision(reason="bf16 onrm"):
                    for nt in range(NC4):
                        nc.vector.tensor_scalar_mul(
                            onrm[:, Dh * nt:Dh * (nt + 1)],
                            p3[:, nt, 0:Dh], recip4[:, nt:nt + 1])

            def emit_pv_fin(s, h):
                """Transpose normalized O[n, d] back to outT[d, n] via PE."""
                onrm = onrms[s][h]
                potT = ps.tile([Dh, N], BF16, tag="psT", name=f"potT{s}_{h}", bufs=1)
                for nt in range(NC4):
                    nc.tensor.transpose(potT[:, 128 * nt:128 * (nt + 1)],
                                        onrm[:, Dh * nt:Dh * (nt + 1)], identb[:])
                outT = get(outTs[s], h // 2, lambda: sb.tile(
                    [128, N], BF16, tag="outT", name=f"outT_s{s}_{h // 2}", bufs=12))
                hb = 64 * (h % 2)
                nc.vector.tensor_copy(outT[hb:hb + 64, :], potT[:])

            def emit_proj(s, n4):
                outT = outTs[s]
                if s == 1 and n4 == NC4 - 1:
                    # final unit: 6 narrow psum groups across queues so the
                    # drain pipelines and the last bytes leave ASAP
                    osb = sb.tile([128, C], BF16, tag="osb", name=f"osb{s}_{n4}", bufs=2)
                    for g in range(6):
                        c0 = 128 * g
                        pr = ps.tile([128, 128], F32, tag="ps1", name=f"pr{s}_{n4}_{g}", bufs=3)
                        for cc in range(CC6):
                            lhsT = outT[cc][:, 128 * n4:128 * (n4 + 1)]
                            nc.tensor.matmul(pr[:], lhsT, projw[cc][:, c0:c0 + 128],
                                             start=(cc == 0), stop=(cc == CC6 - 1))
                        eng = (nc.scalar.copy, nc.vector.tensor_copy)[g % 2]
                        eng(osb[:, c0:c0 + 128], pr[:])
                        deng = (nc.sync, nc.gpsimd)[g % 2]
                        deng.dma_start(y[s, 128 * n4:128 * (n4 + 1), c0:c0 + 128],
                                       osb[:, c0:c0 + 128])
                    return
                if s == 1:
                    # the score pair-ring (ps2) is idle during the s1-proj
                    # tail; borrow one 2-bank slot per unit so ps1 stays free
                    # for the last pot drains
                    pr2 = ps.tile([128, 2 * N], F32, tag="ps2", name=f"pr2_{n4}", bufs=2)
                    pra, prb = pr2[:, 0:512], pr2[:, 512:768]
                else:
                    pra = ps.tile([128, 512], F32, tag="ps1", name=f"pra{s}_{n4}", bufs=3)[:]
                    prb = ps.tile([128, 256], F32, tag="ps1", name=f"prb{s}_{n4}", bufs=3)[:]
                # pra group completes first so its copy overlaps the prb group
                for cc in range(CC6):
                    nc.tensor.matmul(pra[:], outT[cc][:, 128 * n4:128 * (n4 + 1)],
                                     projw[cc][:, 0:512],
                                     start=(cc == 0), stop=(cc == CC6 - 1))
                for cc in range(CC6):
                    nc.tensor.matmul(prb[:], outT[cc][:, 128 * n4:128 * (n4 + 1)],
                                     projw[cc][:, 512:768],
                                     start=(cc == 0), stop=(cc == CC6 - 1))
                osb = sb.tile([128, C], BF16, tag="osb", name=f"osb{s}_{n4}", bufs=2)
                ceng = nc.vector.tensor_copy if s == 0 else nc.scalar.copy
                ceng(osb[:, 0:512], pra[:])
                nc.sync.dma_start(y[s, 128 * n4:128 * (n4 + 1), 0:512], osb[:, 0:512])
                ceng2 = nc.vector.tensor_copy if s == 0 else nc.scalar.copy
                ceng2(osb[:, 512:768], prb[:])
                nc.gpsimd.dma_start(y[s, 128 * n4:128 * (n4 + 1), 512:768], osb[:, 512:768])

            # ---- interleaved schedule; PV lags scores by one head ----
            for cc in range(CC6):
                emit_x_dma(0, cc)
            emit_weight_dmas()
            for m4 in range(NC4):
                emit_v(0, m4)
            # 8 of 12 s0 qk chunks up front; 4 deferred into the heads phase
            for jc in (0, 6, 1, 7, 2, 8, 3, 9):
                emit_qk(0, jc)
            for cc in range(CC6):
                emit_x_dma(1, cc)
            for m4 in range(NC4):
                emit_v(1, m4)
            # s0 attention; deferred s0 qk + all s1 qk as PE filler (1/head)
            e1 = [(emit_qk, 0, 4), (emit_qk, 0, 10), (emit_qk, 0, 5), (emit_qk, 0, 11),
                  (emit_qk, 1, 0), (emit_qk, 1, 6), (emit_qk, 1, 1), (emit_qk, 1, 7),
                  (emit_qk, 1, 2), (emit_qk, 1, 8), (emit_qk, 1, 3), (emit_qk, 1, 9)]
            k = 0
            for h in range(H):
                emit_scores(0, h)
                if h > 0:
                    emit_pv_mm(0, h - 1)
                if h > 1:
                    emit_pv_fin(0, h - 2)
                if h == 3:
                    emit_projw_dmas()
                # keep the last two filler units for the slice boundary,
                # where the ps2 ring waits on the final exps
                tgt = min(len(e1) - 1, (len(e1) * (h + 1) + H - 1) // H)
                while k < tgt:
                    f, a, b = e1[k]; f(a, b); k += 1
            emit_pv_mm(0, H - 1)
            emit_pv_fin(0, H - 2)
            while k < len(e1):
                f, a, b = e1[k]; f(a, b); k += 1
            emit_pv_fin(0, H - 1)
            # s1 attention; s0 proj + deferred s1 qk as filler
            e2 = [(emit_proj, 0, 0), (emit_qk, 1, 4), (emit_qk, 1, 10),
                  (emit_proj, 0, 1), (emit_qk, 1, 5), (emit_qk, 1, 11),
                  (emit_proj, 0, 2), (emit_proj, 0, 3)]
            k = 0
            for h in range(H):
                emit_scores(1, h)
                if h > 0:
                    emit_pv_mm(1, h - 1)
                if h > 1:
                    emit_pv_fin(1, h - 2)
                tgt = min(len(e2), (len(e2) * (h + 1) + H - 1) // H)
                while k < tgt:
                    f, a, b = e2[k]; f(a, b); k += 1
            emit_pv_mm(1, H - 1)
            emit_pv_fin(1, H - 2)
            emit_pv_fin(1, H - 1)
            for n4 in range(NC4):
                emit_proj(1, n4)

    nc.finalize()
    return nc


def _host_prep(x, mask, qkv_w, proj_w):
    scale = Dh ** -0.5
    qkv_wT = np.ascontiguousarray(qkv_w.T).astype(np.float32)
    qkv_wT[:, :C] *= scale
    bf = ml_dtypes.bfloat16
    qkv_wTqk = np.ascontiguousarray(qkv_wT[:, :2 * C]).astype(bf)
    qkv_wTv = np.ascontiguousarray(qkv_wT[:, 2 * C:]).astype(bf)
    proj_wT = np.ascontiguousarray(proj_w.T).astype(bf)
    em = np.exp(np.ascontiguousarray(
        mask.reshape(N, N).T).astype(np.float32)).astype(bf)
    # (2, 128, 1024): block a = [E[256a : 256a+128] | E[256a+128 : 256a+256]]
    emaskT2 = np.concatenate(
        [np.concatenate([em[256 * a:256 * a + 128], em[256 * a + 128:256 * a + 256]],
                        axis=1)[None] for a in range(2)], axis=0).reshape(2 * 128, 2 * N)
    xT = np.ascontiguousarray(
        x.reshape(B * T, N, C).transpose(0, 2, 1)).astype(bf)   # (16, C, N)
    return xT, qkv_wTqk, qkv_wTv, proj_wT, emaskT2


def sim_feed(inputs):
    """Feed dict for a single-core CoreSim run (slices 0-1)."""
    x, mask = np.asarray(inputs["x"]), np.asarray(inputs["mask"])
    qkv_w, proj_w = np.asarray(inputs["qkv_w"]), np.asarray(inputs["proj_w"])
    xT, qkv_wTqk, qkv_wTv, proj_wT, emaskT2 = _host_prep(x, mask, qkv_w, proj_w)
    return {"xTs": xT[0:SL], "qkv_wTqk": qkv_wTqk, "qkv_wTv": qkv_wTv,
            "proj_wT": proj_wT, "emaskT2": emaskT2}


def kernel(x, mask, qkv_w, q_bias, v_bias, proj_w, proj_b, _trace=False, _trace_kwargs=None):
    x, mask, qkv_w, proj_w = (np.asarray(a) for a in (x, mask, qkv_w, proj_w))
    q_bias, v_bias, proj_b = (np.asarray(a) for a in (q_bias, v_bias, proj_b))
    # biases folded in host-side only if nonzero (spec: all zeros). Assert to be safe.
    assert not np.any(q_bias) and not np.any(v_bias) and not np.any(proj_b), \
        "nonzero biases not supported by this kernel build"
    xT, qkv_wTqk, qkv_wTv, proj_wT, emaskT2 = _host_prep(x, mask, qkv_w, proj_w)

    if "nc" not in _cache:
        _cache["nc"] = build_nc()
    nc = _cache["nc"]

    in_maps = []
    for c in range(NCORES):
        in_maps.append({
            "xTs": xT[SL * c:SL * (c + 1)],
            "qkv_wTqk": qkv_wTqk,
            "qkv_wTv": qkv_wTv,
            "proj_wT": proj_wT,
            "emaskT2": emaskT2,
        })
    res = run_bass_kernel_spmd(
        nc, in_maps, core_ids=list(range(NCORES)),
        trace=_trace, **(_trace_kwargs or {}),
    )
    out = np.concatenate([np.asarray(res.results[c]["y"]).astype(np.float32)
                          for c in range(NCORES)], axis=0)
    out = out.reshape(B, T, N, C)
    if _trace:
        return out, res
    return out


# revision 34
# speedup vs baseline: 1.0089x; 1.0089x over previous
"""Trainium2 Bass kernel for nn_Attention (B=2,T=8,N=512,C=768,H=12).

Strategy: data-parallel over the 16 (b,t) slices -> 2 slices per core, 8 cores.
All transposes and dtype conversion done on host (free). On-chip per slice:
  xT[c, n] arrives pre-transposed, bf16
  qkT[d, n] = W_qk @ xT      (bf16; scale folded into Wq on host)
  v[m, (h, dh+1)] = xT.T @ W_v  with ones column per head (bf16)
  ST[m, n] = kT.T @ qT       (bf16; no mask on PE; two m-chunks share one
                              [128,1024] PSUM pair-tile)
  P0 = exp(ST)               (Act engine, 2 wide exps per head)
  P = P0 * exp(maskT)        (multiplicative mask, Pool, SBUF bf16)
  O[n, (dh+1)] per n-tile = P.T-chunks @ v  (flipped PV, ap=65 bf16
                              matmuls; col dh accumulates l via ones column)
  Onorm = O * 1/l            (per-partition tensor_scalar on DVE)
  outT[d, n] = transpose(Onorm) on PE; y = outT.T @ proj_wT (bf16)
Attention pipeline is software-pipelined: scores(h) | PV(h-1) | transpose(h-2)
so the PE never waits on the Act exp or DVE normalize round-trips; qkv/proj
units are interleaved as PE filler throughout both slices' head phases.
"""
import sys

sys.path.insert(0, "/opt/trn_rl_repo")

import numpy as np
import ml_dtypes
import concourse.bacc as bacc
import concourse.mybir as mybir
import concourse.tile as tile
from concourse.bass_utils import run_bass_kernel_spmd
from concourse.masks import make_identity

B, T, N, C = 2, 8, 512, 768
H = 12
Dh = C // H            # 64
SL = 2                 # slices per core
NCORES = 8
NC4 = N // 128         # 4 n-chunks
CC6 = C // 128         # 6 c-chunks
F32 = mybir.dt.float32
BF16 = mybir.dt.bfloat16

_cache = {}


def build_nc():
    nc = bacc.Bacc()
    xTs = nc.dram_tensor("xTs", [SL, C, N], BF16, kind="ExternalInput")
    qkv_wTqk = nc.dram_tensor("qkv_wTqk", [C, 2 * C], BF16, kind="ExternalInput")
    qkv_wTv = nc.dram_tensor("qkv_wTv", [C, C], BF16, kind="ExternalInput")
    proj_wT = nc.dram_tensor("proj_wT", [C, C], BF16, kind="ExternalInput")
    # exp(mask.T) with m-chunk pairs side by side: row-block a holds
    # [E[256a:256a+128], E[256a+128:256a+256]] as a (128, 1024) tile
    emaskT2 = nc.dram_tensor("emaskT2", [2 * 128, 2 * N], BF16, kind="ExternalInput")
    y = nc.dram_tensor("y", [SL, N, C], BF16, kind="ExternalOutput")

    with tile.TileContext(nc) as tc:
        with (
            tc.tile_pool(name="wpool", bufs=1) as wpool,
            tc.tile_pool(name="sb", bufs=1) as sb,
            tc.tile_pool(name="ps", bufs=1, space="PSUM") as ps,
        ):
            # ---- persistent weights ----
            qkw = [wpool.tile([128, 2 * C], BF16, tag=f"qkw{cc}", name=f"qkw{cc}") for cc in range(CC6)]
            vw = [wpool.tile([128, C], BF16, tag=f"vw{cc}", name=f"vw{cc}") for cc in range(CC6)]
            projw = [wpool.tile([128, C], BF16, tag=f"projw{cc}", name=f"projw{cc}") for cc in range(CC6)]
            emask2 = [wpool.tile([128, 2 * N], BF16, tag=f"emask{a}", name=f"emask{a}") for a in range(2)]

            def emit_weight_dmas():
                # xT+vw interleaved across all three DMA queues (the v phase
                # needs every chunk of both); qkw behind them; emask last
                for cc in range(CC6):
                    eng = (nc.gpsimd, nc.sync, nc.scalar)[cc % 3]
                    eng.dma_start(vw[cc][:], qkv_wTv[128 * cc:128 * (cc + 1), :])
                for cc in range(CC6):
                    eng = nc.sync if cc < 3 else nc.gpsimd
                    eng.dma_start(qkw[cc][:], qkv_wTqk[128 * cc:128 * (cc + 1), :])
                for a in range(2):
                    nc.gpsimd.dma_start(emask2[a][:], emaskT2[128 * a:128 * (a + 1), :])

            def emit_projw_dmas():
                for cc in range(CC6):
                    nc.gpsimd.dma_start(projw[cc][:], proj_wT[128 * cc:128 * (cc + 1), :])

            onesf = wpool.tile([128, Dh], F32, tag="onesf")
            nc.gpsimd.memset(onesf[:], 1.0)
            identf = wpool.tile([128, 128], F32, tag="identf")
            make_identity(nc, identf[:])
            identb = wpool.tile([128, 128], BF16, tag="identb")
            with nc.allow_low_precision(reason="bf16 identity"):
                nc.vector.tensor_copy(identb[:], identf[:])

            xTt = [[None] * CC6 for _ in range(SL)]
            vsbs = [[None] * NC4 for _ in range(SL)]
            qks = [[None] * (2 * CC6) for _ in range(SL)]
            outTs = [[None] * CC6 for _ in range(SL)]
            ptss = [[None] * H for _ in range(SL)]   # per-head [ptA, ptB]

            def get(lst, i, mk):
                if lst[i] is None:
                    lst[i] = mk()
                return lst[i]

            def emit_x_dma(s, cc):
                xT = get(xTt[s], cc, lambda cc=cc: sb.tile(
                    [128, N], BF16, tag="xT", name=f"xT_s{s}_{cc}", bufs=12))
                if s == 0:
                    eng = (nc.sync, nc.scalar, nc.gpsimd)[cc % 3]
                else:
                    eng = nc.sync if cc % 2 == 0 else nc.gpsimd
                eng.dma_start(xT[:], xTs[s, 128 * cc:128 * (cc + 1), :])

            def emit_v(s, m4):
                xT = xTt[s]
                vsb = get(vsbs[s], m4, lambda: sb.tile(
                    [128, H * (Dh + 1)], BF16, tag="vsb", name=f"vsb_s{s}_{m4}", bufs=8))
                pva = ps.tile([128, 512], F32, tag="ps1", name=f"pva{s}_{m4}", bufs=3)
                pvb = ps.tile([128, 256], F32, tag="ps1", name=f"pvb{s}_{m4}", bufs=3)
                # pva group completes first so its copy overlaps the pvb group
                for i in range(CC6):
                    cc = (m4 + i) % CC6
                    nc.tensor.matmul(pva[:], xT[cc][:, 128 * m4:128 * (m4 + 1)],
                                     vw[cc][:, 0:512],
                                     start=(i == 0), stop=(i == CC6 - 1))
                for i in range(CC6):
                    cc = (m4 + i) % CC6
                    nc.tensor.matmul(pvb[:], xT[cc][:, 128 * m4:128 * (m4 + 1)],
                                     vw[cc][:, 512:768],
                                     start=(i == 0), stop=(i == CC6 - 1))
                v3 = vsb[:].rearrange("p (h e) -> p h e", e=Dh + 1)
                cpy = nc.scalar.copy
                with nc.allow_low_precision(reason="bf16 v"):
                    cpy(v3[:, 0:8, 0:Dh], pva[:].rearrange("p (h e) -> p h e", e=Dh))
                    cpy(v3[:, 8:12, 0:Dh], pvb[:].rearrange("p (h e) -> p h e", e=Dh))
                    nc.vector.tensor_copy(v3[:, :, Dh:Dh + 1],
                                          onesf[:, 0:H].rearrange("p (h e) -> p h e", e=1))

            def emit_qk(s, jc):
                xT = xTt[s]
                qkt = get(qks[s], jc, lambda: sb.tile(
                    [128, N], BF16, tag="qk", name=f"qk_s{s}_{jc}", bufs=13))
                pqk = ps.tile([128, N], F32, tag="ps1", name=f"pqk{s}_{jc}", bufs=3)
                for i in range(CC6):
                    cc = (jc + i) % CC6
                    nc.tensor.matmul(pqk[:], qkw[cc][:, 128 * jc:128 * (jc + 1)], xT[cc][:],
                                     start=(i == 0), stop=(i == CC6 - 1))
                cpy = nc.vector.tensor_copy if jc % 2 == 0 else nc.scalar.copy
                with nc.allow_low_precision(reason="bf16 qk"):
                    cpy(qkt[:], pqk[:])

            def emit_scores(s, h):
                """Scores for head h: two [128,1024] PSUM pair-tiles, each
                covering two m-chunks; one exp + one mask-mul per pair."""
                qk = qks[s]
                hb = 64 * (h % 2)
                qTh = qk[h // 2][hb:hb + 64, :]
                kTh = qk[CC6 + h // 2][hb:hb + 64, :]
                pts = []
                for a in range(2):
                    pst2 = ps.tile([128, 2 * N], F32, tag="ps2", name=f"pst{s}_{h}_{a}", bufs=2)
                    p02 = sb.tile([128, 2 * N], BF16, tag="p0", name=f"p0_{s}_{h}_{a}", bufs=8)
                    pt2 = sb.tile([128, 2 * N], BF16, tag="pt", name=f"pt{s}_{h}_{a}", bufs=8)
                    for b in range(2):
                        mc = 2 * a + b
                        nc.tensor.matmul(pst2[:, 512 * b:512 * (b + 1)],
                                         kTh[:, 128 * mc:128 * (mc + 1)], qTh,
                                         start=True, stop=True)
                    with nc.allow_low_precision(reason="bf16 P"):
                        nc.scalar.activation(p02[:], pst2[:],
                                             mybir.ActivationFunctionType.Exp)
                        # Pool is the lightest engine; DVE carries the
                        # normalize/copy chain
                        nc.gpsimd.tensor_mul(pt2[:], p02[:], emask2[a][:])
                    pts.append(pt2)
                ptss[s][h] = pts

            onrms = [[None] * H for _ in range(SL)]

            def emit_pv_mm(s, h):
                """Flipped PV: out[n, dh+1] per n-tile (ap=65 matmuls), then
                per-partition softmax normalize into SBUF bf16."""
                vsb = vsbs[s]
                pts = ptss[s][h]
                pot = ps.tile([128, NC4 * (Dh + 1)], F32, tag="ps1",
                              name=f"pot{s}_{h}", bufs=3)
                p3 = pot[:].rearrange("p (t e) -> p t e", e=Dh + 1)
                for nt in range(NC4):
                    for mc in range(NC4):
                        lhsT = pts[mc // 2][:, 512 * (mc % 2) + 128 * nt:
                                            512 * (mc % 2) + 128 * (nt + 1)]
                        nc.tensor.matmul(p3[:, nt, :], lhsT,
                                         vsb[mc][:, (Dh + 1) * h:(Dh + 1) * (h + 1)],
                                         start=(mc == 0), stop=(mc == NC4 - 1))
                recip4 = sb.tile([128, NC4], F32, tag="recip", name=f"recip{s}_{h}", bufs=4)
                r3 = recip4[:].rearrange("p (t e) -> p t e", e=1)
                nc.vector.reciprocal(r3[:], p3[:, :, Dh:Dh + 1])
                onrm = get(onrms[s], h, lambda: sb.tile(
                    [128, NC4 * Dh], BF16, tag="onrm", name=f"onrm{s}_{h}", bufs=4))
                with nc.allow_low_precision(reason="bf16 onrm"):
                    for nt in range(NC4):
                        nc.vector.tensor_scalar_mul(
                            onrm[:, Dh * nt:Dh * (nt + 1)],
                            p3[:, nt, 0:Dh], recip4[:, nt:nt + 1])

            def emit_pv_fin(s, h):
                """Transpose normalized O[n, d] back to outT[d, n] via PE."""
                onrm = onrms[s][h]
                potT = ps.tile([Dh, N], BF16, tag="psT", name=f"potT{s}_{h}", bufs=1)
                for nt in range(NC4):
                    nc.tensor.transpose(potT[:, 128 * nt:128 * (nt + 1)],
                                        onrm[:, Dh * nt:Dh * (nt + 1)], identb[:])
                outT = get(outTs[s], h // 2, lambda: sb.tile(
                    [128, N], BF16, tag="outT", name=f"outT_s{s}_{h // 2}", bufs=12))
                hb = 64 * (h % 2)
                nc.vector.tensor_copy(outT[hb:hb + 64, :], potT[:])

            def emit_proj(s, n4):
                outT = outTs[s]
                if s == 1 and n4 == NC4 - 1:
                    # final unit: 6 narrow psum groups across queues so the
                    # drain pipelines and the last bytes leave ASAP
                    osb = sb.tile([128, C], BF16, tag="osb", name=f"osb{s}_{n4}", bufs=2)
                    for g in range(6):
                        c0 = 128 * g
                        pr = ps.tile([128, 128], F32, tag="ps1", name=f"pr{s}_{n4}_{g}", bufs=3)
                        for cc in range(CC6):
                            lhsT = outT[cc][:, 128 * n4:128 * (n4 + 1)]
                            nc.tensor.matmul(pr[:], lhsT, projw[cc][:, c0:c0 + 128],
                                             start=(cc == 0), stop=(cc == CC6 - 1))
                        eng = (nc.scalar.copy, nc.vector.tensor_copy)[g % 2]
                        eng(osb[:, c0:c0 + 128], pr[:])
                        deng = (nc.sync, nc.gpsimd)[g % 2]
                        deng.dma_start(y[s, 128 * n4:128 * (n4 + 1), c0:c0 + 128],
                                       osb[:, c0:c0 + 128])
                    return
                pra = ps.tile([128, 512], F32, tag="ps1", name=f"pra{s}_{n4}", bufs=3)[:]
                prb = ps.tile([128, 256], F32, tag="ps1", name=f"prb{s}_{n4}", bufs=3)[:]
                # pra group completes first so its copy overlaps the prb group
                for cc in range(CC6):
                    nc.tensor.matmul(pra[:], outT[cc][:, 128 * n4:128 * (n4 + 1)],
                                     projw[cc][:, 0:512],
                                     start=(cc == 0), stop=(cc == CC6 - 1))
                for cc in range(CC6):
                    nc.tensor.matmul(prb[:], outT[cc][:, 128 * n4:128 * (n4 + 1)],
                                     projw[cc][:, 512:768],
                                     start=(cc == 0), stop=(cc == CC6 - 1))
                osb = sb.tile([128, C], BF16, tag="osb", name=f"osb{s}_{n4}", bufs=2)
                ceng = nc.vector.tensor_copy if s == 0 else nc.scalar.copy
                ceng(osb[:, 0:512], pra[:])
                nc.sync.dma_start(y[s, 128 * n4:128 * (n4 + 1), 0:512], osb[:, 0:512])
                ceng2 = nc.vector.tensor_copy if s == 0 else nc.scalar.copy
                ceng2(osb[:, 512:768], prb[:])
                nc.gpsimd.dma_start(y[s, 128 * n4:128 * (n4 + 1), 512:768], osb[:, 512:768])

            # ---- interleaved schedule; PV lags scores by one head ----
            for cc in range(CC6):
                emit_x_dma(0, cc)
            emit_weight_dmas()
            for m4 in range(NC4):
                emit_v(0, m4)
            # 8 of 12 s0 qk chunks up front; 4 deferred into the heads phase
            for jc in (0, 6, 1, 7, 2, 8, 3, 9):
                emit_qk(0, jc)
            for cc in range(CC6):
                emit_x_dma(1, cc)
            for m4 in range(NC4):
                emit_v(1, m4)
            # s0 attention; deferred s0 qk + all s1 qk as PE filler (1/head)
            e1 = [(emit_qk, 0, 4), (emit_qk, 0, 10), (emit_qk, 0, 5), (emit_qk, 0, 11),
                  (emit_qk, 1, 0), (emit_qk, 1, 6), (emit_qk, 1, 1), (emit_qk, 1, 7),
                  (emit_qk, 1, 2), (emit_qk, 1, 8), (emit_qk, 1, 3), (emit_qk, 1, 9)]
            k = 0
            for h in range(H):
                emit_scores(0, h)
                if h > 0:
                    emit_pv_mm(0, h - 1)
                if h > 1:
                    emit_pv_fin(0, h - 2)
                if h == 3:
                    emit_projw_dmas()
                # keep the last two filler units for the slice boundary,
                # where the ps2 ring waits on the final exps
                tgt = min(len(e1) - 1, (len(e1) * (h + 1) + H - 1) // H)
                while k < tgt:
                    f, a, b = e1[k]; f(a, b); k += 1
            emit_pv_mm(0, H - 1)
            emit_pv_fin(0, H - 2)
            while k < len(e1):
                f, a, b = e1[k]; f(a, b); k += 1
            emit_pv_fin(0, H - 1)
            # s1 attention; s0 proj + deferred s1 qk as filler
            e2 = [(emit_proj, 0, 0), (emit_qk, 1, 4), (emit_qk, 1, 10),
                  (emit_proj, 0, 1), (emit_qk, 1, 5), (emit_qk, 1, 11),
                  (emit_proj, 0, 2), (emit_proj, 0, 3)]
            k = 0
            for h in range(H):
                emit_scores(1, h)
                if h > 0:
                    emit_pv_mm(1, h - 1)
                if h > 1:
                    emit_pv_fin(1, h - 2)
                tgt = min(len(e2), (len(e2) * (h + 1) + H - 1) // H)
                while k < tgt:
                    f, a, b = e2[k]; f(a, b); k += 1
            emit_pv_mm(1, H - 1)
            emit_pv_fin(1, H - 2)
            emit_pv_fin(1, H - 1)
            for n4 in range(NC4):
                emit_proj(1, n4)

    nc.finalize()
    return nc


def _host_prep(x, mask, qkv_w, proj_w):
    scale = Dh ** -0.5
    qkv_wT = np.ascontiguousarray(qkv_w.T).astype(np.float32)
    qkv_wT[:, :C] *= scale
    bf = ml_dtypes.bfloat16
    qkv_wTqk = np.ascontiguousarray(qkv_wT[:, :2 * C]).astype(bf)
    qkv_wTv = np.ascontiguousarray(qkv_wT[:, 2 * C:]).astype(bf)
    proj_wT = np.ascontiguousarray(proj_w.T).astype(bf)
    em = np.exp(np.ascontiguousarray(
        mask.reshape(N, N).T).astype(np.float32)).astype(bf)
    # (2, 128, 1024): block a = [E[256a : 256a+128] | E[256a+128 : 256a+256]]
    emaskT2 = np.concatenate(
        [np.concatenate([em[256 * a:256 * a + 128], em[256 * a + 128:256 * a + 256]],
                        axis=1)[None] for a in range(2)], axis=0).reshape(2 * 128, 2 * N)
    xT = np.ascontiguousarray(
        x.reshape(B * T, N, C).transpose(0, 2, 1)).astype(bf)   # (16, C, N)
    return xT, qkv_wTqk, qkv_wTv, proj_wT, emaskT2


def sim_feed(inputs):
    """Feed dict for a single-core CoreSim run (slices 0-1)."""
    x, mask = np.asarray(inputs["x"]), np.asarray(inputs["mask"])
    qkv_w, proj_w = np.asarray(inputs["qkv_w"]), np.asarray(inputs["proj_w"])
    xT, qkv_wTqk, qkv_wTv, proj_wT, emaskT2 = _host_prep(x, mask, qkv_w, proj_w)
    return {"xTs": xT[0:SL], "qkv_wTqk": qkv_wTqk, "qkv_wTv": qkv_wTv,
            "proj_wT": proj_wT, "emaskT2": emaskT2}


def kernel(x, mask, qkv_w, q_bias, v_bias, proj_w, proj_b, _trace=False, _trace_kwargs=None):
    x, mask, qkv_w, proj_w = (np.asarray(a) for a in (x, mask, qkv_w, proj_w))
    q_bias, v_bias, proj_b = (np.asarray(a) for a in (q_bias, v_bias, proj_b))
    # biases folded in host-side only if nonzero (spec: all zeros). Assert to be safe.
    assert not np.any(q_bias) and not np.any(v_bias) and not np.any(proj_b), \
        "nonzero biases not supported by this kernel build"
    xT, qkv_wTqk, qkv_wTv, proj_wT, emaskT2 = _host_prep(x, mask, qkv_w, proj_w)

    if "nc" not in _cache:
        _cache["nc"] = build_nc()
    nc = _cache["nc"]

    in_maps = []
    for c in range(NCORES):
        in_maps.append({
            "xTs": xT[SL * c:SL * (c + 1)],
            "qkv_wTqk": qkv_wTqk,
            "qkv_wTv": qkv_wTv,
            "proj_wT": proj_wT,
            "emaskT2": emaskT2,
        })
    res = run_bass_kernel_spmd(
        nc, in_maps, core_ids=list(range(NCORES)),
        trace=_trace, **(_trace_kwargs or {}),
    )
    out = np.concatenate([np.asarray(res.results[c]["y"]).astype(np.float32)
                          for c in range(NCORES)], axis=0)
    out = out.reshape(B, T, N, C)
    if _trace:
        return out, res
    return out


# revision 35
# speedup vs baseline: 1.0111x; 1.0022x over previous
"""Trainium2 Bass kernel for nn_Attention (B=2,T=8,N=512,C=768,H=12).

Strategy: data-parallel over the 16 (b,t) slices -> 2 slices per core, 8 cores.
All transposes and dtype conversion done on host (free). On-chip per slice:
  xT[c, n] arrives pre-transposed, bf16
  qkT[d, n] = W_qk @ xT      (bf16; scale folded into Wq on host)
  v[m, (h, dh+1)] = xT.T @ W_v  with ones column per head (bf16)
  ST[m, n] = kT.T @ qT       (bf16; no mask on PE; two m-chunks share one
                              [128,1024] PSUM pair-tile)
  P0 = exp(ST)               (Act engine, 2 wide exps per head)
  P = P0 * exp(maskT)        (multiplicative mask, Pool, SBUF bf16)
  O[n, (dh+1)] per n-tile = P.T-chunks @ v  (flipped PV, ap=65 bf16
                              matmuls; col dh accumulates l via ones column)
  Onorm = O * 1/l            (per-partition tensor_scalar on DVE)
  outT[d, n] = transpose(Onorm) on PE; y = outT.T @ proj_wT (bf16)
Attention pipeline is software-pipelined: scores(h) | PV(h-1) | transpose(h-2)
so the PE never waits on the Act exp or DVE normalize round-trips; qkv/proj
units are interleaved as PE filler throughout both slices' head phases.
"""
import sys

sys.path.insert(0, "/opt/trn_rl_repo")

import numpy as np
import ml_dtypes
import concourse.bacc as bacc
import concourse.mybir as mybir
import concourse.tile as tile
from concourse.bass_utils import run_bass_kernel_spmd
from concourse.masks import make_identity

B, T, N, C = 2, 8, 512, 768
H = 12
Dh = C // H            # 64
SL = 2                 # slices per core
NCORES = 8
NC4 = N // 128         # 4 n-chunks
CC6 = C // 128         # 6 c-chunks
F32 = mybir.dt.float32
BF16 = mybir.dt.bfloat16

_cache = {}


def build_nc():
    nc = bacc.Bacc()
    xTs = nc.dram_tensor("xTs", [SL, C, N], BF16, kind="ExternalInput")
    qkv_wTqk = nc.dram_tensor("qkv_wTqk", [C, 2 * C], BF16, kind="ExternalInput")
    qkv_wTv = nc.dram_tensor("qkv_wTv", [C, C], BF16, kind="ExternalInput")
    proj_wT = nc.dram_tensor("proj_wT", [C, C], BF16, kind="ExternalInput")
    # exp(mask.T) with m-chunk pairs side by side: row-block a holds
    # [E[256a:256a+128], E[256a+128:256a+256]] as a (128, 1024) tile
    emaskT2 = nc.dram_tensor("emaskT2", [2 * 128, 2 * N], BF16, kind="ExternalInput")
    y = nc.dram_tensor("y", [SL, N, C], BF16, kind="ExternalOutput")

    with tile.TileContext(nc) as tc:
        with (
            tc.tile_pool(name="wpool", bufs=1) as wpool,
            tc.tile_pool(name="sb", bufs=1) as sb,
            tc.tile_pool(name="ps", bufs=1, space="PSUM") as ps,
        ):
            # ---- persistent weights ----
            qkw = [wpool.tile([128, 2 * C], BF16, tag=f"qkw{cc}", name=f"qkw{cc}") for cc in range(CC6)]
            vw = [wpool.tile([128, C], BF16, tag=f"vw{cc}", name=f"vw{cc}") for cc in range(CC6)]
            projw = [wpool.tile([128, C], BF16, tag=f"projw{cc}", name=f"projw{cc}") for cc in range(CC6)]
            emask2 = [wpool.tile([128, 2 * N], BF16, tag=f"emask{a}", name=f"emask{a}") for a in range(2)]

            def emit_weight_dmas():
                # xT+vw interleaved across all three DMA queues (the v phase
                # needs every chunk of both); qkw behind them; emask last
                for cc in range(CC6):
                    eng = (nc.gpsimd, nc.sync, nc.scalar)[cc % 3]
                    eng.dma_start(vw[cc][:], qkv_wTv[128 * cc:128 * (cc + 1), :])
                for cc in range(CC6):
                    eng = nc.sync if cc < 3 else nc.gpsimd
                    eng.dma_start(qkw[cc][:], qkv_wTqk[128 * cc:128 * (cc + 1), :])
                for a in range(2):
                    nc.gpsimd.dma_start(emask2[a][:], emaskT2[128 * a:128 * (a + 1), :])

            def emit_projw_dmas():
                for cc in range(CC6):
                    nc.gpsimd.dma_start(projw[cc][:], proj_wT[128 * cc:128 * (cc + 1), :])

            onesf = wpool.tile([128, Dh], F32, tag="onesf")
            nc.gpsimd.memset(onesf[:], 1.0)
            identf = wpool.tile([128, 128], F32, tag="identf")
            make_identity(nc, identf[:])
            identb = wpool.tile([128, 128], BF16, tag="identb")
            with nc.allow_low_precision(reason="bf16 identity"):
                nc.vector.tensor_copy(identb[:], identf[:])

            xTt = [[None] * CC6 for _ in range(SL)]
            vsbs = [[None] * NC4 for _ in range(SL)]
            qks = [[None] * (2 * CC6) for _ in range(SL)]
            outTs = [[None] * CC6 for _ in range(SL)]
            ptss = [[None] * H for _ in range(SL)]   # per-head [ptA, ptB]

            def get(lst, i, mk):
                if lst[i] is None:
                    lst[i] = mk()
                return lst[i]

            def emit_x_dma(s, cc):
                xT = get(xTt[s], cc, lambda cc=cc: sb.tile(
                    [128, N], BF16, tag="xT", name=f"xT_s{s}_{cc}", bufs=12))
                if s == 0:
                    eng = (nc.sync, nc.scalar, nc.gpsimd)[cc % 3]
                else:
                    eng = nc.sync if cc % 2 == 0 else nc.gpsimd
                eng.dma_start(xT[:], xTs[s, 128 * cc:128 * (cc + 1), :])

            def emit_v(s, m4):
                xT = xTt[s]
                vsb = get(vsbs[s], m4, lambda: sb.tile(
                    [128, H * (Dh + 1)], BF16, tag="vsb", name=f"vsb_s{s}_{m4}", bufs=8))
                pva = ps.tile([128, 512], F32, tag="ps1", name=f"pva{s}_{m4}", bufs=3)
                pvb = ps.tile([128, 256], F32, tag="ps1", name=f"pvb{s}_{m4}", bufs=3)
                # pva group completes first so its copy overlaps the pvb group
                for i in range(CC6):
                    cc = (m4 + i) % CC6
                    nc.tensor.matmul(pva[:], xT[cc][:, 128 * m4:128 * (m4 + 1)],
                                     vw[cc][:, 0:512],
                                     start=(i == 0), stop=(i == CC6 - 1))
                for i in range(CC6):
                    cc = (m4 + i) % CC6
                    nc.tensor.matmul(pvb[:], xT[cc][:, 128 * m4:128 * (m4 + 1)],
                                     vw[cc][:, 512:768],
                                     start=(i == 0), stop=(i == CC6 - 1))
                v3 = vsb[:].rearrange("p (h e) -> p h e", e=Dh + 1)
                cpy = nc.scalar.copy
                with nc.allow_low_precision(reason="bf16 v"):
                    cpy(v3[:, 0:8, 0:Dh], pva[:].rearrange("p (h e) -> p h e", e=Dh))
                    cpy(v3[:, 8:12, 0:Dh], pvb[:].rearrange("p (h e) -> p h e", e=Dh))
                    nc.vector.tensor_copy(v3[:, :, Dh:Dh + 1],
                                          onesf[:, 0:H].rearrange("p (h e) -> p h e", e=1))

            def emit_qk(s, jc):
                xT = xTt[s]
                qkt = get(qks[s], jc, lambda: sb.tile(
                    [128, N], BF16, tag="qk", name=f"qk_s{s}_{jc}", bufs=13))
                pqk = ps.tile([128, N], F32, tag="ps1", name=f"pqk{s}_{jc}", bufs=3)
                for i in range(CC6):
                    cc = (jc + i) % CC6
                    nc.tensor.matmul(pqk[:], qkw[cc][:, 128 * jc:128 * (jc + 1)], xT[cc][:],
                                     start=(i == 0), stop=(i == CC6 - 1))
                deferred = (s == 1) or jc in (4, 5, 10, 11)
                cpy = nc.vector.tensor_copy if (deferred or jc % 2 == 0) else nc.scalar.copy
                with nc.allow_low_precision(reason="bf16 qk"):
                    cpy(qkt[:], pqk[:])

            def emit_scores(s, h):
                """Scores for head h: two [128,1024] PSUM pair-tiles, each
                covering two m-chunks; one exp + one mask-mul per pair."""
                qk = qks[s]
                hb = 64 * (h % 2)
                qTh = qk[h // 2][hb:hb + 64, :]
                kTh = qk[CC6 + h // 2][hb:hb + 64, :]
                pts = []
                for a in range(2):
                    pst2 = ps.tile([128, 2 * N], F32, tag="ps2", name=f"pst{s}_{h}_{a}", bufs=2)
                    p02 = sb.tile([128, 2 * N], BF16, tag="p0", name=f"p0_{s}_{h}_{a}", bufs=8)
                    pt2 = sb.tile([128, 2 * N], BF16, tag="pt", name=f"pt{s}_{h}_{a}", bufs=8)
                    for b in range(2):
                        mc = 2 * a + b
                        nc.tensor.matmul(pst2[:, 512 * b:512 * (b + 1)],
                                         kTh[:, 128 * mc:128 * (mc + 1)], qTh,
                                         start=True, stop=True)
                    with nc.allow_low_precision(reason="bf16 P"):
                        nc.scalar.activation(p02[:], pst2[:],
                                             mybir.ActivationFunctionType.Exp)
                        # Pool is the lightest engine; DVE carries the
                        # normalize/copy chain
                        nc.gpsimd.tensor_mul(pt2[:], p02[:], emask2[a][:])
                    pts.append(pt2)
                ptss[s][h] = pts

            onrms = [[None] * H for _ in range(SL)]

            def emit_pv_mm(s, h):
                """Flipped PV: out[n, dh+1] per n-tile (ap=65 matmuls), then
                per-partition softmax normalize into SBUF bf16."""
                vsb = vsbs[s]
                pts = ptss[s][h]
                pot = ps.tile([128, NC4 * (Dh + 1)], F32, tag="ps1",
                              name=f"pot{s}_{h}", bufs=3)
                p3 = pot[:].rearrange("p (t e) -> p t e", e=Dh + 1)
                for nt in range(NC4):
                    for mc in range(NC4):
                        lhsT = pts[mc // 2][:, 512 * (mc % 2) + 128 * nt:
                                            512 * (mc % 2) + 128 * (nt + 1)]
                        nc.tensor.matmul(p3[:, nt, :], lhsT,
                                         vsb[mc][:, (Dh + 1) * h:(Dh + 1) * (h + 1)],
                                         start=(mc == 0), stop=(mc == NC4 - 1))
                recip4 = sb.tile([128, NC4], F32, tag="recip", name=f"recip{s}_{h}", bufs=4)
                r3 = recip4[:].rearrange("p (t e) -> p t e", e=1)
                nc.vector.reciprocal(r3[:], p3[:, :, Dh:Dh + 1])
                onrm = get(onrms[s], h, lambda: sb.tile(
                    [128, NC4 * Dh], BF16, tag="onrm", name=f"onrm{s}_{h}", bufs=4))
                with nc.allow_low_precision(reason="bf16 onrm"):
                    for nt in range(NC4):
                        nc.vector.tensor_scalar_mul(
                            onrm[:, Dh * nt:Dh * (nt + 1)],
                            p3[:, nt, 0:Dh], recip4[:, nt:nt + 1])

            def emit_pv_fin(s, h):
                """Transpose normalized O[n, d] back to outT[d, n] via PE."""
                onrm = onrms[s][h]
                potT = ps.tile([Dh, N], BF16, tag="psT", name=f"potT{s}_{h}", bufs=1)
                for nt in range(NC4):
                    nc.tensor.transpose(potT[:, 128 * nt:128 * (nt + 1)],
                                        onrm[:, Dh * nt:Dh * (nt + 1)], identb[:])
                outT = get(outTs[s], h // 2, lambda: sb.tile(
                    [128, N], BF16, tag="outT", name=f"outT_s{s}_{h // 2}", bufs=12))
                hb = 64 * (h % 2)
                ceng = nc.scalar.copy if (s == 1 and h >= 9) else nc.vector.tensor_copy
                ceng(outT[hb:hb + 64, :], potT[:])

            def emit_proj(s, n4):
                outT = outTs[s]
                if s == 1 and n4 == NC4 - 1:
                    # final unit: 6 narrow psum groups across queues so the
                    # drain pipelines and the last bytes leave ASAP
                    osb = sb.tile([128, C], BF16, tag="osb", name=f"osb{s}_{n4}", bufs=2)
                    for g in range(6):
                        c0 = 128 * g
                        pr = ps.tile([128, 128], F32, tag="ps1", name=f"pr{s}_{n4}_{g}", bufs=3)
                        for cc in range(CC6):
                            lhsT = outT[cc][:, 128 * n4:128 * (n4 + 1)]
                            nc.tensor.matmul(pr[:], lhsT, projw[cc][:, c0:c0 + 128],
                                             start=(cc == 0), stop=(cc == CC6 - 1))
                        eng = (nc.scalar.copy, nc.vector.tensor_copy)[g % 2]
                        eng(osb[:, c0:c0 + 128], pr[:])
                        deng = (nc.sync, nc.gpsimd)[g % 2]
                        deng.dma_start(y[s, 128 * n4:128 * (n4 + 1), c0:c0 + 128],
                                       osb[:, c0:c0 + 128])
                    return
                pra = ps.tile([128, 512], F32, tag="ps1", name=f"pra{s}_{n4}", bufs=3)[:]
                prb = ps.tile([128, 256], F32, tag="ps1", name=f"prb{s}_{n4}", bufs=3)[:]
                # pra group completes first so its copy overlaps the prb group
                for cc in range(CC6):
                    nc.tensor.matmul(pra[:], outT[cc][:, 128 * n4:128 * (n4 + 1)],
                                     projw[cc][:, 0:512],
                                     start=(cc == 0), stop=(cc == CC6 - 1))
                for cc in range(CC6):
                    nc.tensor.matmul(prb[:], outT[cc][:, 128 * n4:128 * (n4 + 1)],
                                     projw[cc][:, 512:768],
                                     start=(cc == 0), stop=(cc == CC6 - 1))
                osb = sb.tile([128, C], BF16, tag="osb", name=f"osb{s}_{n4}", bufs=2)
                ceng = nc.vector.tensor_copy if s == 0 else nc.scalar.copy
                ceng(osb[:, 0:512], pra[:])
                nc.sync.dma_start(y[s, 128 * n4:128 * (n4 + 1), 0:512], osb[:, 0:512])
                ceng2 = nc.vector.tensor_copy if s == 0 else nc.scalar.copy
                ceng2(osb[:, 512:768], prb[:])
                nc.gpsimd.dma_start(y[s, 128 * n4:128 * (n4 + 1), 512:768], osb[:, 512:768])

            # ---- interleaved schedule; PV lags scores by one head ----
            for cc in range(CC6):
                emit_x_dma(0, cc)
            emit_weight_dmas()
            for m4 in range(NC4):
                emit_v(0, m4)
            # 8 of 12 s0 qk chunks up front; 4 deferred into the heads phase
            for jc in (0, 6, 1, 7, 2, 8, 3, 9):
                emit_qk(0, jc)
            for cc in range(CC6):
                emit_x_dma(1, cc)
            for m4 in range(NC4):
                emit_v(1, m4)
            # s0 attention; deferred s0 qk + all s1 qk as PE filler (1/head)
            e1 = [(emit_qk, 0, 4), (emit_qk, 0, 10), (emit_qk, 0, 5), (emit_qk, 0, 11),
                  (emit_qk, 1, 0), (emit_qk, 1, 6), (emit_qk, 1, 1), (emit_qk, 1, 7),
                  (emit_qk, 1, 2), (emit_qk, 1, 8), (emit_qk, 1, 3), (emit_qk, 1, 9)]
            k = 0
            for h in range(H):
                emit_scores(0, h)
                if h > 0:
                    emit_pv_mm(0, h - 1)
                if h > 1:
                    emit_pv_fin(0, h - 2)
                if h == 3:
                    emit_projw_dmas()
                # keep the last two filler units for the slice boundary,
                # where the ps2 ring waits on the final exps
                tgt = min(len(e1) - 1, (len(e1) * (h + 1) + H - 1) // H)
                while k < tgt:
                    f, a, b = e1[k]; f(a, b); k += 1
            emit_pv_mm(0, H - 1)
            emit_pv_fin(0, H - 2)
            while k < len(e1):
                f, a, b = e1[k]; f(a, b); k += 1
            emit_pv_fin(0, H - 1)
            # s1 attention; s0 proj + deferred s1 qk as filler
            e2 = [(emit_proj, 0, 0), (emit_qk, 1, 4), (emit_qk, 1, 10),
                  (emit_proj, 0, 1), (emit_qk, 1, 5), (emit_qk, 1, 11),
                  (emit_proj, 0, 2), (emit_proj, 0, 3)]
            k = 0
            for h in range(H):
                emit_scores(1, h)
                if h > 0:
                    emit_pv_mm(1, h - 1)
                if h > 1:
                    emit_pv_fin(1, h - 2)
                tgt = min(len(e2), (len(e2) * (h + 1) + H - 1) // H)
                while k < tgt:
                    f, a, b = e2[k]; f(a, b); k += 1
            emit_pv_mm(1, H - 1)
            emit_pv_fin(1, H - 2)
            emit_pv_fin(1, H - 1)
            for n4 in range(NC4):
                emit_proj(1, n4)

    nc.finalize()
    return nc


def _host_prep(x, mask, qkv_w, proj_w):
    scale = Dh ** -0.5
    qkv_wT = np.ascontiguousarray(qkv_w.T).astype(np.float32)
    qkv_wT[:, :C] *= scale
    bf = ml_dtypes.bfloat16
    qkv_wTqk = np.ascontiguousarray(qkv_wT[:, :2 * C]).astype(bf)
    qkv_wTv = np.ascontiguousarray(qkv_wT[:, 2 * C:]).astype(bf)
    proj_wT = np.ascontiguousarray(proj_w.T).astype(bf)
    em = np.exp(np.ascontiguousarray(
        mask.reshape(N, N).T).astype(np.float32)).astype(bf)
    # (2, 128, 1024): block a = [E[256a : 256a+128] | E[256a+128 : 256a+256]]
    emaskT2 = np.concatenate(
        [np.concatenate([em[256 * a:256 * a + 128], em[256 * a + 128:256 * a + 256]],
                        axis=1)[None] for a in range(2)], axis=0).reshape(2 * 128, 2 * N)
    xT = np.ascontiguousarray(
        x.reshape(B * T, N, C).transpose(0, 2, 1)).astype(bf)   # (16, C, N)
    return xT, qkv_wTqk, qkv_wTv, proj_wT, emaskT2


def sim_feed(inputs):
    """Feed dict for a single-core CoreSim run (slices 0-1)."""
    x, mask = np.asarray(inputs["x"]), np.asarray(inputs["mask"])
    qkv_w, proj_w = np.asarray(inputs["qkv_w"]), np.asarray(inputs["proj_w"])
    xT, qkv_wTqk, qkv_wTv, proj_wT, emaskT2 = _host_prep(x, mask, qkv_w, proj_w)
    return {"xTs": xT[0:SL], "qkv_wTqk": qkv_wTqk, "qkv_wTv": qkv_wTv,
            "proj_wT": proj_wT, "emaskT2": emaskT2}


def kernel(x, mask, qkv_w, q_bias, v_bias, proj_w, proj_b, _trace=False, _trace_kwargs=None):
    x, mask, qkv_w, proj_w = (np.asarray(a) for a in (x, mask, qkv_w, proj_w))
    q_bias, v_bias, proj_b = (np.asarray(a) for a in (q_bias, v_bias, proj_b))
    # biases folded in host-side only if nonzero (spec: all zeros). Assert to be safe.
    assert not np.any(q_bias) and not np.any(v_bias) and not np.any(proj_b), \
        "nonzero biases not supported by this kernel build"
    xT, qkv_wTqk, qkv_wTv, proj_wT, emaskT2 = _host_prep(x, mask, qkv_w, proj_w)

    if "nc" not in _cache:
        _cache["nc"] = build_nc()
    nc = _cache["nc"]

    in_maps = []
    for c in range(NCORES):
        in_maps.append({
            "xTs": xT[SL * c:SL * (c + 1)],
            "qkv_wTqk": qkv_wTqk,
            "qkv_wTv": qkv_wTv,
            "proj_wT": proj_wT,
            "emaskT2": emaskT2,
        })
    res = run_bass_kernel_spmd(
        nc, in_maps, core_ids=list(range(NCORES)),
        trace=_trace, **(_trace_kwargs or {}),
    )
    out = np.concatenate([np.asarray(res.results[c]["y"]).astype(np.float32)
                          for c in range(NCORES)], axis=0)
    out = out.reshape(B, T, N, C)
    if _trace:
        return out, res
    return out
